# revision 11
# baseline (speedup 1.0000x reference)
"""Additive attention (Bahdanau) Trainium2 kernel, 8-core data parallel.

out = softmax_k(mask(sum_h w_v[h] * tanh(qf[q,h] + kf[k,h]))) @ V
with qf = q@Wq, kf = k@Wk.

Instead of materializing tanh over the [nq, nk, H] feature tensor (16.8M
ACT elements per core -> ~110us floor), tanh is expanded in a sine series

    tanh(s) ~ sum_m beta_m sin(m*w0*s),   s = qf + kf

and each sin(m*w0*(a+b)) = sin(m*w0*a)cos(m*w0*b) + cos(m*w0*a)sin(m*w0*b)
is SEPARABLE: the score reduction over h becomes 2 matmuls per harmonic on
the (otherwise idle) PE.  Only the small projected tensors qf [64,256] and
kf [512,256] ever pass through trig:

  ACT:  sin/cos fundamentals (args kept in [-pi,pi] via a clamp fused into
        the PSUM evacuation) + squares, then the softmax exps
        (2 table loads total: trig_and_small, exp_and_others)
  DVE:  harmonic ladder in bf16 (double-angle + angle-addition products;
        scale factors from halved products folded into beta / TS imms)
  PE :  projections, 32 accumulating score matmuls per batch, attn @ V

Harmonics {1,2,3,4,5,6,8,10}, period 2P = 21 (w0 = 2pi/21), clamp 5.2.
Numpy-simulated end-to-end rel err vs the exact reference: 5.0e-3
(gate 2e-2).
"""

import os
from contextlib import ExitStack

import ml_dtypes
import numpy as np

import concourse.bacc as bacc
import concourse.bass as bass
import concourse.mybir as mybir
import concourse.tile as tile
from concourse.bass_utils import run_bass_kernel_spmd

F32 = mybir.dt.float32
BF16 = mybir.dt.bfloat16
I32 = mybir.dt.int32
AF = mybir.ActivationFunctionType
ALU = mybir.AluOpType

B, NQ, NK, QS, KS, H, VD = 16, 64, 512, 256, 256, 256, 256
NCORES = 8
BPC = B // NCORES  # batches per core
MASK_NEG = -30.0

PI = float(np.pi)
W0 = 2.0 * PI / 21.0       # fundamental frequency (P = 10.5)
CLAMP = 5.2                # |qf|,|kf| clamp so all ACT sin args stay in range
HSET = [1, 2, 3, 4, 5, 6, 8, 10]
# ridge fit of tanh on [-10.5, 10.5], gaussian(sigma=sqrt(2)) + 1e-3 floor
# weighting, with the ladder's halved-product factors folded in
# (s4,s6,s10 stored as sin/2 -> x2; s8 stored as sin/4 -> x4)
BETA_F = [1.24833822, -0.03631956, 0.36988800, -0.16552149,
          0.18117062, -0.00469805, 0.14800902, 0.02661305]


def _build():
    nc = bacc.Bacc()
    q_d = nc.declare_dram_parameter("queries", [BPC, NQ, QS], F32, isOutput=False)
    k_d = nc.declare_dram_parameter("keys", [BPC, NK, KS], F32, isOutput=False)
    v_d = nc.declare_dram_parameter("values", [BPC, NK, VD], F32, isOutput=False)
    vl_d = nc.declare_dram_parameter("valid_lens", [BPC, 1], I32, isOutput=False)
    wq_d = nc.declare_dram_parameter("W_q", [QS, H], F32, isOutput=False)
    wk_d = nc.declare_dram_parameter("W_k", [KS, H], F32, isOutput=False)
    wv_d = nc.declare_dram_parameter("w_v", [H], F32, isOutput=False)
    out_d = nc.declare_dram_parameter("out", [BPC, NQ, VD], F32, isOutput=True)

    identb_d = nc.inline_tensor(
        np.eye(128).astype(ml_dtypes.bfloat16), name="identb_c"
    )
    krow_d = nc.inline_tensor(np.arange(NK, dtype=np.float32), name="krow_c")

    with ExitStack() as ctx:
        tc = ctx.enter_context(tile.TileContext(nc))
        consts = ctx.enter_context(tc.tile_pool(name="consts", bufs=1))
        setup = ctx.enter_context(tc.tile_pool(name="setup", bufs=2))
        lad = ctx.enter_context(tc.tile_pool(name="lad", bufs=2))
        qlad = ctx.enter_context(tc.tile_pool(name="qlad", bufs=1))
        sm = ctx.enter_context(tc.tile_pool(name="sm", bufs=1))
        outp = ctx.enter_context(tc.tile_pool(name="outp", bufs=2))
        ps_sc = ctx.enter_context(tc.tile_pool(name="ps_sc", bufs=2, space="PSUM"))
        ps_misc = ctx.enter_context(tc.tile_pool(name="ps_misc", bufs=2, space="PSUM"))
        ps_out = ctx.enter_context(tc.tile_pool(name="ps_out", bufs=2, space="PSUM"))

        # ---------------- loads (critical first, two queues) ----------------
        k_sb0 = setup.tile([128, 4, KS], F32, tag="k_sb0", bufs=1)
        k0_view = k_d[0].rearrange("(kb p) d -> p kb d", p=128)
        identb = consts.tile([128, 128], BF16)
        nc.sync.dma_start(out=k_sb0[:, 0], in_=k0_view[:, 0])
        nc.gpsimd.dma_start(out=identb, in_=identb_d[:, :])
        nc.gpsimd.dma_start(out=k_sb0[:, 1], in_=k0_view[:, 1])
        nc.sync.dma_start(out=k_sb0[:, 2], in_=k0_view[:, 2])
        nc.gpsimd.dma_start(out=k_sb0[:, 3], in_=k0_view[:, 3])
        wk_sb = setup.tile([128, 2, H], F32, tag="wk_f", bufs=1)
        nc.gpsimd.dma_start(out=wk_sb, in_=wk_d.rearrange("(kt p) m -> p kt m", p=128))
        wq_sb = setup.tile([128, 2, H], F32, tag="wq_f", bufs=1)
        nc.sync.dma_start(out=wq_sb, in_=wq_d.rearrange("(kt p) m -> p kt m", p=128))
        q_sb0 = setup.tile([NQ, QS], F32, tag="q_sb0", bufs=1)
        nc.sync.dma_start(out=q_sb0, in_=q_d[0])
        q_sb1 = setup.tile([NQ, QS], F32, tag="q_sb1", bufs=1)
        nc.sync.dma_start(out=q_sb1, in_=q_d[1])
        wv_col = consts.tile([128, 2], F32)
        nc.gpsimd.dma_start(out=wv_col, in_=wv_d.rearrange("(t p) -> p t", p=128))
        k_sb1 = setup.tile([128, 4, KS], F32, tag="k_sb1", bufs=1)
        k1_view = k_d[1].rearrange("(kb p) d -> p kb d", p=128)
        for kb in range(4):
            eng = nc.sync if kb % 2 == 0 else nc.gpsimd
            eng.dma_start(out=k_sb1[:, kb], in_=k1_view[:, kb])
        krow = consts.tile([128, NK], F32)
        nc.sync.dma_start(out=krow, in_=krow_d[None, :].partition_broadcast(128))
        v_sbs, valid_sbs = [], []
        for b in range(BPC):
            v_sb = setup.tile([128, 4, VD], F32, tag=f"v_sb{b}", name=f"v_sb{b}", bufs=1)
            nc.gpsimd.dma_start(
                out=v_sb, in_=v_d[b].rearrange("(kb p) d -> p kb d", p=128)
            )
            v_sbs.append(v_sb)
            valid_sb = setup.tile([128, 1], I32, tag=f"valid{b}", name=f"valid{b}")
            nc.gpsimd.dma_start(
                out=valid_sb, in_=vl_d[b : b + 1, :].partition_broadcast(128)
            )
            valid_sbs.append(valid_sb)

        k_sbs = [k_sb0, k_sb1]
        q_sbs = [q_sb0, q_sb1]

        # ACT bias constants (per-partition APs)
        halfpi = consts.tile([128, 1], F32)
        nc.gpsimd.memset(halfpi, PI / 2)

        # weights to bf16 (first on the DVE stream)
        wq_bf = consts.tile([128, 2, H], BF16)
        wk_bf = consts.tile([128, 2, H], BF16)
        for kt in range(2):
            nc.vector.tensor_copy(out=wk_bf[:, kt], in_=wk_sb[:, kt])
            nc.vector.tensor_copy(out=wq_bf[:, kt], in_=wq_sb[:, kt])

        # ---------------- transposes + projections ----------------
        def transpose_project_k(b):
            """bf16 cast -> bf16 PE transposes -> projection; DVE evac + clamp."""
            k_bf = setup.tile([128, 4, KS], BF16, tag="k_bf", name=f"k_bf{b}")
            nc.vector.tensor_copy(out=k_bf, in_=k_sbs[b])
            kT_bf = setup.tile([128, 2, NK], BF16, tag="kT", name=f"kT{b}")
            for kb in range(4):
                for kt in range(2):
                    pst = ps_misc.tile([128, 1024], BF16, tag="ps_miscb", name="pst_k")
                    nc.tensor.transpose(
                        pst[:, 0:128], k_bf[:, kb, kt * 128 : (kt + 1) * 128], identb
                    )
                    nc.vector.tensor_copy(
                        out=kT_bf[:, kt, kb * 128 : (kb + 1) * 128], in_=pst[:, 0:128]
                    )
            kfT = setup.tile([128, 2, NK], F32, tag="kfT", name=f"kfT{b}")
            for mt in range(2):
                psp = ps_misc.tile([128, 512], F32, tag="ps_misc", name="psp_k")
                for kt in range(2):
                    nc.tensor.matmul(
                        psp,
                        lhsT=wk_bf[:, kt, mt * 128 : (mt + 1) * 128],
                        rhs=kT_bf[:, kt, :],
                        start=(kt == 0),
                        stop=(kt == 1),
                    )
                # evacuate + clamp to +-CLAMP in one DVE op
                nc.vector.tensor_scalar(
                    out=kfT[:, mt], in0=psp, scalar1=CLAMP, scalar2=-CLAMP,
                    op0=ALU.min, op1=ALU.max,
                )
            return kfT

        def transpose_project_q():
            """Both batches' q -> combined clamped qfT [128, 2ht, 2b, NQ]."""
            qfT = qlad.tile([128, 2, BPC, NQ], F32, tag="qfT", name="qfT")
            qT_bfs = []
            for b in range(BPC):
                q_bf = setup.tile([NQ, QS], BF16, tag="q_bf", name=f"q_bf{b}")
                nc.vector.tensor_copy(out=q_bf, in_=q_sbs[b])
                qT_bf = setup.tile([128, 2, NQ], BF16, tag="qT", name=f"qT{b}")
                for kt in range(2):
                    pst = ps_misc.tile([128, 1024], BF16, tag="ps_miscb", name="pst_q")
                    nc.tensor.transpose(
                        pst[:, 0:NQ],
                        q_bf[:, kt * 128 : (kt + 1) * 128],
                        identb[0:NQ, 0:NQ],
                    )
                    nc.vector.tensor_copy(out=qT_bf[:, kt, :], in_=pst[:, 0:NQ])
                qT_bfs.append(qT_bf)
            for b in range(BPC):
                for mt in range(2):
                    psp = ps_misc.tile([128, 512], F32, tag="ps_misc", name="psp_q")
                    for kt in range(2):
                        nc.tensor.matmul(
                            psp[:, 0:NQ],
                            lhsT=wq_bf[:, kt, mt * 128 : (mt + 1) * 128],
                            rhs=qT_bfs[b][:, kt, :],
                            start=(kt == 0),
                            stop=(kt == 1),
                        )
                    nc.vector.tensor_scalar(
                        out=qfT[:, mt, b], in0=psp[:, 0:NQ],
                        scalar1=CLAMP, scalar2=-CLAMP,
                        op0=ALU.min, op1=ALU.max,
                    )
            return qfT

        # ---------------- trig ladder ----------------
        def act_fundamentals(x, pool, shape, pref):
            """ACT: s1, c1, s2, s1^2, s2^2 of x (f32 in, bf16 out)."""
            t = {}
            for nm in ("s1", "c1", "s2", "s1sq", "s2sq"):
                t[nm] = pool.tile(shape, BF16, tag=nm, name=f"{pref}_{nm}")
            nc.scalar.activation(out=t["s1"], in_=x, func=AF.Sin, scale=W0)
            nc.scalar.activation(out=t["c1"], in_=x, func=AF.Sin, scale=-W0,
                                 bias=halfpi[:, 0:1])
            nc.scalar.activation(out=t["s2"], in_=x, func=AF.Sin, scale=2 * W0)
            nc.scalar.activation(out=t["s1sq"], in_=t["s1"], func=AF.Square)
            nc.scalar.activation(out=t["s2sq"], in_=t["s2"], func=AF.Square)
            return t

        def dve_ladder(t, pool, shape, pref):
            """DVE bf16 ladder -> full trig dict {m: (sin-ish, cos-ish)}.

            s4,s6,s10 hold sin/2; s8 holds sin/4 (folded into BETA_F / imms).
            """
            def tl(nm, tag=None):
                return pool.tile(shape, BF16, tag=tag or nm, name=f"{pref}_{nm}")
            TT, TS = nc.vector.tensor_tensor, nc.vector.tensor_scalar
            t["c2"] = tl("c2")
            TS(out=t["c2"], in0=t["s1sq"], scalar1=-2.0, scalar2=1.0,
               op0=ALU.mult, op1=ALU.add)
            p3a = tl("p3a", tag="p3")
            TS(out=p3a, in0=t["s1sq"], scalar1=-4.0, scalar2=3.0,
               op0=ALU.mult, op1=ALU.add)
            t["s3"] = tl("s3")
            TT(out=t["s3"], in0=t["s1"], in1=p3a, op=ALU.mult)
            p3b = tl("p3b", tag="p3")
            TS(out=p3b, in0=t["s1sq"], scalar1=-4.0, scalar2=1.0,
               op0=ALU.mult, op1=ALU.add)
            t["c3"] = tl("c3")
            TT(out=t["c3"], in0=t["c1"], in1=p3b, op=ALU.mult)
            t["c4"] = tl("c4")
            TS(out=t["c4"], in0=t["s2sq"], scalar1=-2.0, scalar2=1.0,
               op0=ALU.mult, op1=ALU.add)
            t["s4"] = tl("s4")
            TT(out=t["s4"], in0=t["s2"], in1=t["c2"], op=ALU.mult)   # sin4/2
            ta, tb = tl("ta", tag="t0"), tl("tb", tag="t1")
            TT(out=ta, in0=t["s2"], in1=t["c3"], op=ALU.mult)
            TT(out=tb, in0=t["c2"], in1=t["s3"], op=ALU.mult)
            t["s5"] = tl("s5")
            TT(out=t["s5"], in0=ta, in1=tb, op=ALU.add)
            tc_, td = tl("tc", tag="t0"), tl("td", tag="t1")
            TT(out=tc_, in0=t["c2"], in1=t["c3"], op=ALU.mult)
            TT(out=td, in0=t["s2"], in1=t["s3"], op=ALU.mult)
            t["c5"] = tl("c5")
            TT(out=t["c5"], in0=tc_, in1=td, op=ALU.subtract)
            t["s6"] = tl("s6")
            TT(out=t["s6"], in0=t["s3"], in1=t["c3"], op=ALU.mult)   # sin6/2
            t["s8"] = tl("s8")
            TT(out=t["s8"], in0=t["s4"], in1=t["c4"], op=ALU.mult)   # sin8/4
            t["s10"] = tl("s10")
            TT(out=t["s10"], in0=t["s5"], in1=t["c5"], op=ALU.mult)  # sin10/2
            return t

        def ladder_highcos(t, pool, shape, pref):
            """ACT squares of s3,s4,s5 then DVE TS -> c6, c8, c10."""
            def tl(nm, tag=None):
                return pool.tile(shape, BF16, tag=tag or nm, name=f"{pref}_{nm}")
            TS = nc.vector.tensor_scalar
            s3sq = tl("s3sq", tag="ssq")
            nc.scalar.activation(out=s3sq, in_=t["s3"], func=AF.Square)
            s4sq = tl("s4sq", tag="ssq2")
            nc.scalar.activation(out=s4sq, in_=t["s4"], func=AF.Square)
            s5sq = tl("s5sq", tag="ssq3")
            nc.scalar.activation(out=s5sq, in_=t["s5"], func=AF.Square)
            t["c6"] = tl("c6")
            TS(out=t["c6"], in0=s3sq, scalar1=-2.0, scalar2=1.0,
               op0=ALU.mult, op1=ALU.add)
            t["c8"] = tl("c8")
            TS(out=t["c8"], in0=s4sq, scalar1=-8.0, scalar2=1.0,
               op0=ALU.mult, op1=ALU.add)
            t["c10"] = tl("c10")
            TS(out=t["c10"], in0=s5sq, scalar1=-2.0, scalar2=1.0,
               op0=ALU.mult, op1=ALU.add)
            return {m: (t[f"s{m}"], t[f"c{m}"]) for m in HSET}

        # ---------------- setup ----------------
        kfT0 = transpose_project_k(0)
        qfT = transpose_project_q()
        kfT1 = transpose_project_k(1)

        # ACT stream: k0, k1, q fundamentals first (squares of the derived
        # harmonics come later so they don't stall the fundamentals)
        kt0 = act_fundamentals(kfT0, lad, [128, 2, NK], "k0")
        kt1 = act_fundamentals(kfT1, lad, [128, 2, NK], "k1")
        qt_ = act_fundamentals(qfT, qlad, [128, 2, BPC, NQ], "q")

        # DVE ladders (k0 first -- its scores start earliest)
        dve_ladder(kt0, lad, [128, 2, NK], "k0")
        dve_ladder(qt_, qlad, [128, 2, BPC, NQ], "q")
        ktrig0 = ladder_highcos(kt0, lad, [128, 2, NK], "k0")
        qtrig = ladder_highcos(qt_, qlad, [128, 2, BPC, NQ], "q")

        # lhs = beta'_m * w_v[h] * trig_q  (both batches at once)
        lhs = {}
        for i, m in enumerate(HSET):
            for fn in (0, 1):  # 0: sin, 1: cos
                lt = qlad.tile([128, 2, BPC, NQ], BF16, tag=f"lhs{m}_{fn}",
                               name=f"lhs{m}_{fn}")
                src = qtrig[m][fn]
                for ht in range(2):
                    nc.vector.tensor_scalar(
                        out=lt[:, ht], in0=src[:, ht],
                        scalar1=wv_col[:, ht : ht + 1], scalar2=BETA_F[i],
                        op0=ALU.mult, op1=ALU.mult,
                    )
                lhs[(m, fn)] = lt

        dve_ladder(kt1, lad, [128, 2, NK], "k1")
        ktrig1 = ladder_highcos(kt1, lad, [128, 2, NK], "k1")

        # ---------------- scores ----------------
        def score_matmuls(b, ktrig, sc_ps):
            first = True
            n = len(HSET)
            for i, m in enumerate(HSET):
                for fn in (0, 1):
                    # sin_q * cos_k  +  cos_q * sin_k
                    rhs = ktrig[m][1 - fn]
                    lt = lhs[(m, fn)]
                    for ht in range(2):
                        nc.tensor.matmul(
                            sc_ps[0:NQ],
                            lhsT=lt[:, ht, b, :],
                            rhs=rhs[:, ht, :],
                            start=first,
                            stop=(i == n - 1 and fn == 1 and ht == 1),
                        )
                        first = False

        # mask biases + v casts built early on GpSimd (idle engine)
        bias_bs, v_bfs = [], []
        for b in range(BPC):
            valid_f = setup.tile([128, 1], F32, tag="validf", name=f"vf{b}")
            nc.gpsimd.tensor_copy(out=valid_f, in_=valid_sbs[b])
            bias_b = setup.tile([128, NK], F32, tag="bias", name=f"bias{b}")
            nc.gpsimd.tensor_scalar(
                out=bias_b, in0=krow, scalar1=valid_f[:, 0:1], scalar2=None,
                op0=ALU.is_lt,
            )
            nc.gpsimd.tensor_scalar(
                out=bias_b, in0=bias_b, scalar1=1.0, scalar2=-MASK_NEG,
                op0=ALU.subtract, op1=ALU.mult,
            )
            bias_bs.append(bias_b)
            v_bf = outp.tile([128, 4, VD], BF16, tag="v_bf", name=f"v_bf{b}")
            for kb in range(4):
                nc.gpsimd.tensor_copy(out=v_bf[:, kb], in_=v_sbs[b][:, kb])
            v_bfs.append(v_bf)

        # ---------------- softmax + output ----------------
        def finish_batch(b, sc_ps):
            sc_sb = sm.tile([NQ, NK], F32, tag=f"scsb{b}", name=f"scsb{b}")
            nc.vector.tensor_tensor(
                out=sc_sb, in0=sc_ps[0:NQ], in1=bias_bs[b][0:NQ], op=ALU.add
            )
            e_sb = sm.tile([NQ, NK], F32, tag=f"e{b}", name=f"e{b}")
            denom = sm.tile([NQ, 1], F32, tag=f"den{b}", name=f"den{b}")
            nc.scalar.activation(out=e_sb, in_=sc_sb, func=AF.Exp, accum_out=denom)
            recip = sm.tile([NQ, 1], F32, tag=f"rec{b}", name=f"rec{b}")
            nc.vector.reciprocal(recip, denom)
            attn = sm.tile([NQ, NK], BF16, tag=f"at{b}", name=f"at{b}")
            nc.vector.tensor_scalar_mul(out=attn, in0=e_sb, scalar1=recip[:, 0:1])

            attnT = outp.tile([128, 4, NQ], BF16, tag="attnT", name=f"attnT{b}")
            for kb in range(4):
                pst = ps_misc.tile([128, 1024], BF16, tag="ps_miscb", name="pst_a")
                nc.tensor.transpose(
                    pst[:, 0:NQ],
                    attn[:, kb * 128 : (kb + 1) * 128],
                    identb[0:NQ, 0:NQ],
                )
                nc.vector.tensor_copy(out=attnT[:, kb], in_=pst[:, 0:NQ])

            po = ps_out.tile([NQ, VD], F32, tag="po", name=f"po{b}")
            for kb in range(4):
                nc.tensor.matmul(
                    po,
                    lhsT=attnT[:, kb],
                    rhs=v_bfs[b][:, kb],
                    start=(kb == 0),
                    stop=(kb == 3),
                )
            o_sb = outp.tile([NQ, VD], F32, tag="o_sb", name=f"o_sb{b}")
            nc.scalar.copy(out=o_sb, in_=po)
            nc.gpsimd.dma_start(out=out_d[b], in_=o_sb)

        sc_ps0 = ps_sc.tile([128, NK], F32, tag="sc", name="sc0")
        score_matmuls(0, ktrig0, sc_ps0)
        # batch 0's softmax/output emitted before batch 1's scores so its
        # PE/DVE work overlaps batch 1's score matmuls
        finish_batch(0, sc_ps0)
        sc_ps1 = ps_sc.tile([128, NK], F32, tag="sc", name="sc1")
        score_matmuls(1, ktrig1, sc_ps1)
        finish_batch(1, sc_ps1)

    nc.compile()
    return nc


_NC_CACHE = None
LAST_RESULTS = None


def kernel(queries, keys, values, valid_lens, W_q, W_k, w_v):
    global _NC_CACHE, LAST_RESULTS
    if _NC_CACHE is None:
        _NC_CACHE = _build()
    nc = _NC_CACHE

    queries = np.ascontiguousarray(queries, dtype=np.float32)
    keys = np.ascontiguousarray(keys, dtype=np.float32)
    values = np.ascontiguousarray(values, dtype=np.float32)
    valid_lens = np.ascontiguousarray(valid_lens, dtype=np.int32)
    W_q = np.ascontiguousarray(W_q, dtype=np.float32)
    W_k = np.ascontiguousarray(W_k, dtype=np.float32)
    w_v = np.ascontiguousarray(w_v, dtype=np.float32)

    in_maps = []
    for c in range(NCORES):
        lo, hi = c * BPC, (c + 1) * BPC
        in_maps.append(
            {
                "queries": queries[lo:hi],
                "keys": keys[lo:hi],
                "values": values[lo:hi],
                "valid_lens": valid_lens[lo:hi].reshape(BPC, 1),
                "W_q": W_q,
                "W_k": W_k,
                "w_v": w_v,
            }
        )

    trace = os.environ.get("ATTN_TRACE", "0") == "1"
    res = run_bass_kernel_spmd(
        nc, in_maps, core_ids=list(range(NCORES)), trace=trace
    )
    LAST_RESULTS = res
    return np.concatenate([r["out"] for r in res.results], axis=0)


# revision 13
# speedup vs baseline: 1.3609x; 1.3609x over previous
"""Additive attention (Bahdanau) Trainium2 kernel, 8-core data parallel.

out = softmax_k(mask(sum_h w_v[h] * tanh(qf[q,h] + kf[k,h]))) @ V
with qf = q@Wq, kf = k@Wk.

Instead of materializing tanh over the [nq, nk, H] feature tensor (16.8M
ACT elements per core -> ~110us floor), tanh is expanded in a sine series

    tanh(s) ~ sum_m beta_m sin(m*w0*s),   s = qf + kf

and each sin(m*w0*(a+b)) = sin(m*w0*a)cos(m*w0*b) + cos(m*w0*a)sin(m*w0*b)
is SEPARABLE: the score reduction over h becomes 2 matmuls per harmonic on
the (otherwise idle) PE.  Only the small projected tensors qf [64,256] and
kf [512,256] ever pass through trig:

  ACT:  sin/cos fundamentals (args kept in [-pi,pi] via a clamp fused into
        the PSUM evacuation) + squares, then the softmax exps
        (2 table loads total: trig_and_small, exp_and_others)
  DVE:  harmonic ladder in bf16 (double-angle + angle-addition products;
        scale factors from halved products folded into beta / TS imms)
  PE :  projections, 32 accumulating score matmuls per batch, attn @ V

Harmonics {1,2,3,4,5,6,8,10}, period 2P = 21 (w0 = 2pi/21), clamp 5.2.
Numpy-simulated end-to-end rel err vs the exact reference: 5.0e-3
(gate 2e-2).
"""

import os
from contextlib import ExitStack

import ml_dtypes
import numpy as np

import concourse.bacc as bacc
import concourse.bass as bass
import concourse.mybir as mybir
import concourse.tile as tile
from concourse.bass_utils import run_bass_kernel_spmd

F32 = mybir.dt.float32
BF16 = mybir.dt.bfloat16
I32 = mybir.dt.int32
AF = mybir.ActivationFunctionType
ALU = mybir.AluOpType

B, NQ, NK, QS, KS, H, VD = 16, 64, 512, 256, 256, 256, 256
NCORES = 8
BPC = B // NCORES  # batches per core
MASK_NEG = -30.0

PI = float(np.pi)
W0 = 2.0 * PI / 21.0       # fundamental frequency (P = 10.5)
CLAMP = 5.2                # |qf|,|kf| clamp so all ACT sin args stay in range
HSET = [1, 2, 3, 4, 5, 6, 8, 10]
# ridge fit of tanh on [-10.5, 10.5], gaussian(sigma=sqrt(2)) + 1e-3 floor
# weighting, with the ladder's halved-product factors folded in
# (s4,s6,s10 stored as sin/2 -> x2; s8 stored as sin/4 -> x4)
BETA_F = [1.24833822, -0.03631956, 0.36988800, -0.16552149,
          0.18117062, -0.00469805, 0.14800902, 0.02661305]


def _build():
    nc = bacc.Bacc()
    q_d = nc.declare_dram_parameter("queries", [BPC, NQ, QS], F32, isOutput=False)
    k_d = nc.declare_dram_parameter("keys", [BPC, NK, KS], F32, isOutput=False)
    v_d = nc.declare_dram_parameter("values", [BPC, NK, VD], F32, isOutput=False)
    vl_d = nc.declare_dram_parameter("valid_lens", [BPC, 1], I32, isOutput=False)
    wq_d = nc.declare_dram_parameter("W_q", [QS, H], F32, isOutput=False)
    wk_d = nc.declare_dram_parameter("W_k", [KS, H], F32, isOutput=False)
    wv_d = nc.declare_dram_parameter("w_v", [H], F32, isOutput=False)
    out_d = nc.declare_dram_parameter("out", [BPC, NQ, VD], F32, isOutput=True)

    identb_d = nc.inline_tensor(
        np.eye(128).astype(ml_dtypes.bfloat16), name="identb_c"
    )
    krow_d = nc.inline_tensor(np.arange(NK, dtype=np.float32), name="krow_c")

    with ExitStack() as ctx:
        tc = ctx.enter_context(tile.TileContext(nc))
        consts = ctx.enter_context(tc.tile_pool(name="consts", bufs=1))
        setup = ctx.enter_context(tc.tile_pool(name="setup", bufs=2))
        lad = ctx.enter_context(tc.tile_pool(name="lad", bufs=2))
        qlad = ctx.enter_context(tc.tile_pool(name="qlad", bufs=1))
        sm = ctx.enter_context(tc.tile_pool(name="sm", bufs=1))
        outp = ctx.enter_context(tc.tile_pool(name="outp", bufs=2))
        ps_sc = ctx.enter_context(tc.tile_pool(name="ps_sc", bufs=2, space="PSUM"))
        ps_misc = ctx.enter_context(tc.tile_pool(name="ps_misc", bufs=2, space="PSUM"))
        ps_out = ctx.enter_context(tc.tile_pool(name="ps_out", bufs=2, space="PSUM"))

        # ---------------- loads (critical first, two queues) ----------------
        k_sb0 = setup.tile([128, 4, KS], F32, tag="k_sb0", bufs=1)
        k0_view = k_d[0].rearrange("(kb p) d -> p kb d", p=128)
        identb = consts.tile([128, 128], BF16)
        nc.sync.dma_start(out=k_sb0[:, 0], in_=k0_view[:, 0])
        nc.gpsimd.dma_start(out=identb, in_=identb_d[:, :])
        nc.gpsimd.dma_start(out=k_sb0[:, 1], in_=k0_view[:, 1])
        nc.sync.dma_start(out=k_sb0[:, 2], in_=k0_view[:, 2])
        nc.gpsimd.dma_start(out=k_sb0[:, 3], in_=k0_view[:, 3])
        wk_sb = setup.tile([128, 2, H], F32, tag="wk_f", bufs=1)
        nc.gpsimd.dma_start(out=wk_sb, in_=wk_d.rearrange("(kt p) m -> p kt m", p=128))
        wq_sb = setup.tile([128, 2, H], F32, tag="wq_f", bufs=1)
        nc.sync.dma_start(out=wq_sb, in_=wq_d.rearrange("(kt p) m -> p kt m", p=128))
        q_sb0 = setup.tile([NQ, QS], F32, tag="q_sb0", bufs=1)
        nc.sync.dma_start(out=q_sb0, in_=q_d[0])
        q_sb1 = setup.tile([NQ, QS], F32, tag="q_sb1", bufs=1)
        nc.sync.dma_start(out=q_sb1, in_=q_d[1])
        wv_col = consts.tile([128, 2], F32)
        nc.gpsimd.dma_start(out=wv_col, in_=wv_d.rearrange("(t p) -> p t", p=128))
        k_sb1 = setup.tile([128, 4, KS], F32, tag="k_sb1", bufs=1)
        k1_view = k_d[1].rearrange("(kb p) d -> p kb d", p=128)
        for kb in range(4):
            eng = nc.sync if kb % 2 == 0 else nc.gpsimd
            eng.dma_start(out=k_sb1[:, kb], in_=k1_view[:, kb])
        krow = consts.tile([128, NK], F32)
        nc.sync.dma_start(out=krow, in_=krow_d[None, :].partition_broadcast(128))
        v_sbs, valid_sbs = [], []
        for b in range(BPC):
            v_sb = setup.tile([128, 4, VD], F32, tag=f"v_sb{b}", name=f"v_sb{b}", bufs=1)
            nc.gpsimd.dma_start(
                out=v_sb, in_=v_d[b].rearrange("(kb p) d -> p kb d", p=128)
            )
            v_sbs.append(v_sb)
            valid_sb = setup.tile([128, 1], I32, tag=f"valid{b}", name=f"valid{b}")
            nc.gpsimd.dma_start(
                out=valid_sb, in_=vl_d[b : b + 1, :].partition_broadcast(128)
            )
            valid_sbs.append(valid_sb)

        k_sbs = [k_sb0, k_sb1]
        q_sbs = [q_sb0, q_sb1]

        # ACT bias constants (per-partition APs)
        halfpi = consts.tile([128, 1], F32)
        nc.gpsimd.memset(halfpi, PI / 2)

        # weights to bf16 (first on the DVE stream)
        wq_bf = consts.tile([128, 2, H], BF16)
        wk_bf = consts.tile([128, 2, H], BF16)
        for kt in range(2):
            nc.vector.tensor_copy(out=wk_bf[:, kt], in_=wk_sb[:, kt])
            nc.vector.tensor_copy(out=wq_bf[:, kt], in_=wq_sb[:, kt])

        # ---------------- transposes + projections ----------------
        def transpose_project_k(b):
            """bf16 cast -> bf16 PE transposes -> projection; DVE evac + clamp."""
            k_bf = setup.tile([128, 4, KS], BF16, tag="k_bf", name=f"k_bf{b}")
            nc.vector.tensor_copy(out=k_bf, in_=k_sbs[b])
            kT_bf = setup.tile([128, 2, NK], BF16, tag="kT", name=f"kT{b}")
            for kb in range(4):
                for kt in range(2):
                    pst = ps_misc.tile([128, 1024], BF16, tag="ps_miscb", name="pst_k")
                    nc.tensor.transpose(
                        pst[:, 0:128], k_bf[:, kb, kt * 128 : (kt + 1) * 128], identb
                    )
                    nc.vector.tensor_copy(
                        out=kT_bf[:, kt, kb * 128 : (kb + 1) * 128], in_=pst[:, 0:128]
                    )
            kfT = setup.tile([128, 2, NK], F32, tag="kfT", name=f"kfT{b}")
            for mt in range(2):
                psp = ps_misc.tile([128, 512], F32, tag="ps_misc", name="psp_k")
                for kt in range(2):
                    nc.tensor.matmul(
                        psp,
                        lhsT=wk_bf[:, kt, mt * 128 : (mt + 1) * 128],
                        rhs=kT_bf[:, kt, :],
                        start=(kt == 0),
                        stop=(kt == 1),
                    )
                # evacuate + clamp to +-CLAMP in one DVE op
                nc.vector.tensor_scalar(
                    out=kfT[:, mt], in0=psp, scalar1=CLAMP, scalar2=-CLAMP,
                    op0=ALU.min, op1=ALU.max,
                )
            return kfT

        def transpose_project_q():
            """Both batches' q -> combined clamped qfT [128, 2ht, 2b, NQ]."""
            qfT = qlad.tile([128, 2, BPC, NQ], F32, tag="qfT", name="qfT")
            qT_bfs = []
            for b in range(BPC):
                q_bf = setup.tile([NQ, QS], BF16, tag="q_bf", name=f"q_bf{b}")
                nc.vector.tensor_copy(out=q_bf, in_=q_sbs[b])
                qT_bf = setup.tile([128, 2, NQ], BF16, tag="qT", name=f"qT{b}")
                for kt in range(2):
                    pst = ps_misc.tile([128, 1024], BF16, tag="ps_miscb", name="pst_q")
                    nc.tensor.transpose(
                        pst[:, 0:NQ],
                        q_bf[:, kt * 128 : (kt + 1) * 128],
                        identb[0:NQ, 0:NQ],
                    )
                    nc.vector.tensor_copy(out=qT_bf[:, kt, :], in_=pst[:, 0:NQ])
                qT_bfs.append(qT_bf)
            for b in range(BPC):
                for mt in range(2):
                    psp = ps_misc.tile([128, 512], F32, tag="ps_misc", name="psp_q")
                    for kt in range(2):
                        nc.tensor.matmul(
                            psp[:, 0:NQ],
                            lhsT=wq_bf[:, kt, mt * 128 : (mt + 1) * 128],
                            rhs=qT_bfs[b][:, kt, :],
                            start=(kt == 0),
                            stop=(kt == 1),
                        )
                    nc.vector.tensor_scalar(
                        out=qfT[:, mt, b], in0=psp[:, 0:NQ],
                        scalar1=CLAMP, scalar2=-CLAMP,
                        op0=ALU.min, op1=ALU.max,
                    )
            return qfT

        # ---------------- trig ladder ----------------
        def act_fundamentals(x, pool, shape, pref):
            """ACT: s1, c1, s2, s1^2, s2^2 of x (f32 in, bf16 out)."""
            t = {}
            for nm in ("s1", "c1", "s2", "s1sq", "s2sq"):
                t[nm] = pool.tile(shape, BF16, tag=nm, name=f"{pref}_{nm}")
            nc.scalar.activation(out=t["s1"], in_=x, func=AF.Sin, scale=W0)
            nc.scalar.activation(out=t["c1"], in_=x, func=AF.Sin, scale=-W0,
                                 bias=halfpi[:, 0:1])
            nc.scalar.activation(out=t["s2"], in_=x, func=AF.Sin, scale=2 * W0)
            nc.scalar.activation(out=t["s1sq"], in_=t["s1"], func=AF.Square)
            nc.scalar.activation(out=t["s2sq"], in_=t["s2"], func=AF.Square)
            return t

        def dve_ladder(t, pool, shape, pref):
            """DVE bf16 ladder -> full trig dict {m: (sin-ish, cos-ish)}.

            s4,s6,s10 hold sin/2; s8 holds sin/4 (folded into BETA_F / imms).
            """
            def tl(nm, tag=None):
                return pool.tile(shape, BF16, tag=tag or nm, name=f"{pref}_{nm}")
            TT, TS = nc.vector.tensor_tensor, nc.vector.tensor_scalar
            t["c2"] = tl("c2")
            TS(out=t["c2"], in0=t["s1sq"], scalar1=-2.0, scalar2=1.0,
               op0=ALU.mult, op1=ALU.add)
            p3a = tl("p3a", tag="p3")
            TS(out=p3a, in0=t["s1sq"], scalar1=-4.0, scalar2=3.0,
               op0=ALU.mult, op1=ALU.add)
            t["s3"] = tl("s3")
            TT(out=t["s3"], in0=t["s1"], in1=p3a, op=ALU.mult)
            p3b = tl("p3b", tag="p3")
            TS(out=p3b, in0=t["s1sq"], scalar1=-4.0, scalar2=1.0,
               op0=ALU.mult, op1=ALU.add)
            t["c3"] = tl("c3")
            TT(out=t["c3"], in0=t["c1"], in1=p3b, op=ALU.mult)
            t["c4"] = tl("c4")
            TS(out=t["c4"], in0=t["s2sq"], scalar1=-2.0, scalar2=1.0,
               op0=ALU.mult, op1=ALU.add)
            t["s4"] = tl("s4")
            TT(out=t["s4"], in0=t["s2"], in1=t["c2"], op=ALU.mult)   # sin4/2
            ta, tb = tl("ta", tag="t0"), tl("tb", tag="t1")
            TT(out=ta, in0=t["s2"], in1=t["c3"], op=ALU.mult)
            TT(out=tb, in0=t["c2"], in1=t["s3"], op=ALU.mult)
            t["s5"] = tl("s5")
            TT(out=t["s5"], in0=ta, in1=tb, op=ALU.add)
            tc_, td = tl("tc", tag="t0"), tl("td", tag="t1")
            TT(out=tc_, in0=t["c2"], in1=t["c3"], op=ALU.mult)
            TT(out=td, in0=t["s2"], in1=t["s3"], op=ALU.mult)
            t["c5"] = tl("c5")
            TT(out=t["c5"], in0=tc_, in1=td, op=ALU.subtract)
            t["s6"] = tl("s6")
            TT(out=t["s6"], in0=t["s3"], in1=t["c3"], op=ALU.mult)   # sin6/2
            t["s8"] = tl("s8")
            TT(out=t["s8"], in0=t["s4"], in1=t["c4"], op=ALU.mult)   # sin8/4
            t["s10"] = tl("s10")
            TT(out=t["s10"], in0=t["s5"], in1=t["c5"], op=ALU.mult)  # sin10/2
            return t

        def ladder_highcos(t, pool, shape, pref):
            """ACT squares of s3,s4,s5 then DVE TS -> c6, c8, c10."""
            def tl(nm, tag=None):
                return pool.tile(shape, BF16, tag=tag or nm, name=f"{pref}_{nm}")
            TS = nc.vector.tensor_scalar
            s3sq = tl("s3sq", tag="ssq")
            nc.scalar.activation(out=s3sq, in_=t["s3"], func=AF.Square)
            s4sq = tl("s4sq", tag="ssq2")
            nc.scalar.activation(out=s4sq, in_=t["s4"], func=AF.Square)
            s5sq = tl("s5sq", tag="ssq3")
            nc.scalar.activation(out=s5sq, in_=t["s5"], func=AF.Square)
            t["c6"] = tl("c6")
            TS(out=t["c6"], in0=s3sq, scalar1=-2.0, scalar2=1.0,
               op0=ALU.mult, op1=ALU.add)
            t["c8"] = tl("c8")
            TS(out=t["c8"], in0=s4sq, scalar1=-8.0, scalar2=1.0,
               op0=ALU.mult, op1=ALU.add)
            t["c10"] = tl("c10")
            TS(out=t["c10"], in0=s5sq, scalar1=-2.0, scalar2=1.0,
               op0=ALU.mult, op1=ALU.add)
            return {m: (t[f"s{m}"], t[f"c{m}"]) for m in HSET}

        # ---------------- setup ----------------
        kfT0 = transpose_project_k(0)
        qfT = transpose_project_q()
        kfT1 = transpose_project_k(1)

        # mask biases + v casts (DVE; gpsimd contends with DVE's SBUF port)
        bias_bs, v_bfs = [], []
        for b in range(BPC):
            valid_f = setup.tile([128, 1], F32, tag="validf", name=f"vf{b}")
            nc.vector.tensor_copy(out=valid_f, in_=valid_sbs[b])
            bias_b = setup.tile([128, NK], F32, tag="bias", name=f"bias{b}")
            nc.vector.tensor_scalar(
                out=bias_b, in0=krow, scalar1=valid_f[:, 0:1], scalar2=None,
                op0=ALU.is_lt,
            )
            nc.vector.tensor_scalar(
                out=bias_b, in0=bias_b, scalar1=1.0, scalar2=-MASK_NEG,
                op0=ALU.subtract, op1=ALU.mult,
            )
            bias_bs.append(bias_b)
            v_bf = outp.tile([128, 4, VD], BF16, tag="v_bf", name=f"v_bf{b}")
            for kb in range(4):
                nc.vector.tensor_copy(out=v_bf[:, kb], in_=v_sbs[b][:, kb])
            v_bfs.append(v_bf)


        # ACT stream: k0, k1, q fundamentals first (squares of the derived
        # harmonics come later so they don't stall the fundamentals)
        kt0 = act_fundamentals(kfT0, lad, [128, 2, NK], "k0")
        kt1 = act_fundamentals(kfT1, lad, [128, 2, NK], "k1")
        qt_ = act_fundamentals(qfT, qlad, [128, 2, BPC, NQ], "q")

        # DVE ladders (k0 first -- its scores start earliest)
        dve_ladder(kt0, lad, [128, 2, NK], "k0")
        dve_ladder(qt_, qlad, [128, 2, BPC, NQ], "q")
        ktrig0 = ladder_highcos(kt0, lad, [128, 2, NK], "k0")
        qtrig = ladder_highcos(qt_, qlad, [128, 2, BPC, NQ], "q")

        # lhs = beta'_m * w_v[h] * trig_q  (both batches at once)
        lhs = {}
        for i, m in enumerate(HSET):
            for fn in (0, 1):  # 0: sin, 1: cos
                lt = qlad.tile([128, 2, BPC, NQ], BF16, tag=f"lhs{m}_{fn}",
                               name=f"lhs{m}_{fn}")
                src = qtrig[m][fn]
                for ht in range(2):
                    nc.vector.tensor_scalar(
                        out=lt[:, ht], in0=src[:, ht],
                        scalar1=wv_col[:, ht : ht + 1], scalar2=BETA_F[i],
                        op0=ALU.mult, op1=ALU.mult,
                    )
                lhs[(m, fn)] = lt

        dve_ladder(kt1, lad, [128, 2, NK], "k1")
        ktrig1 = ladder_highcos(kt1, lad, [128, 2, NK], "k1")

        # ---------------- scores ----------------
        def score_matmuls(b, ktrig, sc_ps):
            first = True
            n = len(HSET)
            for i, m in enumerate(HSET):
                for fn in (0, 1):
                    # sin_q * cos_k  +  cos_q * sin_k
                    rhs = ktrig[m][1 - fn]
                    lt = lhs[(m, fn)]
                    for ht in range(2):
                        nc.tensor.matmul(
                            sc_ps[0:NQ],
                            lhsT=lt[:, ht, b, :],
                            rhs=rhs[:, ht, :],
                            start=first,
                            stop=(i == n - 1 and fn == 1 and ht == 1),
                        )
                        first = False

        # ---------------- softmax + output ----------------
        def finish_batch(b, sc_ps):
            sc_sb = sm.tile([NQ, NK], F32, tag=f"scsb{b}", name=f"scsb{b}")
            nc.vector.tensor_tensor(
                out=sc_sb, in0=sc_ps[0:NQ], in1=bias_bs[b][0:NQ], op=ALU.add
            )
            e_sb = sm.tile([NQ, NK], F32, tag=f"e{b}", name=f"e{b}")
            denom = sm.tile([NQ, 1], F32, tag=f"den{b}", name=f"den{b}")
            nc.scalar.activation(out=e_sb, in_=sc_sb, func=AF.Exp, accum_out=denom)
            recip = sm.tile([NQ, 1], F32, tag=f"rec{b}", name=f"rec{b}")
            nc.vector.reciprocal(recip, denom)
            attn = sm.tile([NQ, NK], BF16, tag=f"at{b}", name=f"at{b}")
            nc.vector.tensor_scalar_mul(out=attn, in0=e_sb, scalar1=recip[:, 0:1])

            attnT = outp.tile([128, 4, NQ], BF16, tag="attnT", name=f"attnT{b}")
            for kb in range(4):
                pst = ps_misc.tile([128, 1024], BF16, tag="ps_miscb", name="pst_a")
                nc.tensor.transpose(
                    pst[:, 0:NQ],
                    attn[:, kb * 128 : (kb + 1) * 128],
                    identb[0:NQ, 0:NQ],
                )
                nc.vector.tensor_copy(out=attnT[:, kb], in_=pst[:, 0:NQ])

            po = ps_out.tile([NQ, VD], F32, tag="po", name=f"po{b}")
            for kb in range(4):
                nc.tensor.matmul(
                    po,
                    lhsT=attnT[:, kb],
                    rhs=v_bfs[b][:, kb],
                    start=(kb == 0),
                    stop=(kb == 3),
                )
            o_sb = outp.tile([NQ, VD], F32, tag="o_sb", name=f"o_sb{b}")
            nc.scalar.copy(out=o_sb, in_=po)
            nc.gpsimd.dma_start(out=out_d[b], in_=o_sb)

        sc_ps0 = ps_sc.tile([128, NK], F32, tag="sc", name="sc0")
        score_matmuls(0, ktrig0, sc_ps0)
        # batch 0's softmax/output emitted before batch 1's scores so its
        # PE/DVE work overlaps batch 1's score matmuls
        finish_batch(0, sc_ps0)
        sc_ps1 = ps_sc.tile([128, NK], F32, tag="sc", name="sc1")
        score_matmuls(1, ktrig1, sc_ps1)
        finish_batch(1, sc_ps1)

    nc.compile()
    return nc


_NC_CACHE = None
LAST_RESULTS = None


def kernel(queries, keys, values, valid_lens, W_q, W_k, w_v):
    global _NC_CACHE, LAST_RESULTS
    if _NC_CACHE is None:
        _NC_CACHE = _build()
    nc = _NC_CACHE

    queries = np.ascontiguousarray(queries, dtype=np.float32)
    keys = np.ascontiguousarray(keys, dtype=np.float32)
    values = np.ascontiguousarray(values, dtype=np.float32)
    valid_lens = np.ascontiguousarray(valid_lens, dtype=np.int32)
    W_q = np.ascontiguousarray(W_q, dtype=np.float32)
    W_k = np.ascontiguousarray(W_k, dtype=np.float32)
    w_v = np.ascontiguousarray(w_v, dtype=np.float32)

    in_maps = []
    for c in range(NCORES):
        lo, hi = c * BPC, (c + 1) * BPC
        in_maps.append(
            {
                "queries": queries[lo:hi],
                "keys": keys[lo:hi],
                "values": values[lo:hi],
                "valid_lens": valid_lens[lo:hi].reshape(BPC, 1),
                "W_q": W_q,
                "W_k": W_k,
                "w_v": w_v,
            }
        )

    trace = os.environ.get("ATTN_TRACE", "0") == "1"
    res = run_bass_kernel_spmd(
        nc, in_maps, core_ids=list(range(NCORES)), trace=trace
    )
    LAST_RESULTS = res
    return np.concatenate([r["out"] for r in res.results], axis=0)
